# revision 26
# baseline (speedup 1.0000x reference)
"""Trainium2 Bass kernel for nn_MemoryWeightedAttention.

out[b,h,q,k] = attention_scores[b,h,q,k] * (1 + 0.066 * mod[b,q])

where mod[b,q] is a small LN/MLP pipeline applied to
(log1p(global_count[id]), tanh-MLP(LN(emb[id]))) -- i.e. mod depends ONLY on
the token id at (b,q) plus a global histogram of input_ids.  So we compute the
full pipeline once over the 1024-entry vocabulary table (in transposed layout:
features on partitions, vocab on the free dim -> no transposes between
layers), then gather per-token scales with one-hot matmuls, and stream the
512MB attention_scores tensor through a per-partition scalar multiply.

v3: streaming is f16 in / f16 out (rel-err budget is 2e-2; f16 costs ~2.4e-4)
which halves HBM traffic; the table pipeline runs in f16 on PE (4x) with the
histogram on DVE (one-hot + row-reduce) and ACT restricted to
Sqrt/Ln/Gelu/Tanh to minimize activation-table reloads; the gather emits
scales in 4 token blocks so the first stream multiplies start before the
whole gather finishes.

Sharding: 8 cores, core i handles batch i//4, heads 4*(i%4) .. 4*(i%4)+4
(a contiguous [8192, 2048] row-slice of the flattened [65536, 2048] scores).
The tiny table pipeline is replicated on every core (no collectives).
"""

import sys

for _p in ("/opt/trn_rl_repo",):
    if _p not in sys.path:
        sys.path.insert(0, _p)

from contextlib import ExitStack

import numpy as np

import concourse.bacc as bacc
import concourse.tile as tile
from concourse import mybir
from concourse.bass import ts
from concourse.bass_utils import run_bass_kernel_spmd
from concourse.masks import make_identity

F32 = mybir.dt.float32
F16 = mybir.dt.float16
AF = mybir.ActivationFunctionType
ALU = mybir.AluOpType
AX = mybir.AxisListType

B, H, S = 2, 16, 2048
CTX = 1024
VD = 288          # valence dim
FFN = 512
EPSILON = 0.066
LN_EPS = 1e-5

N_CORES = 8
HEADS_PER_CORE = H * B // N_CORES          # 4
SHARD_ROWS = HEADS_PER_CORE * S            # 8192
N_TOK = B * S                              # 4096 (global, for counts)
S_TILES = S // 128                         # 16 q tiles per core's batch
CTX_CHUNKS = CTX // 128                    # 8
ROWS_PER_CHUNK = 512                       # score rows per DMA chunk
N_CHUNKS = SHARD_ROWS // ROWS_PER_CHUNK    # 16
SUB_TILES = ROWS_PER_CHUNK // 128          # 4
GBLK = 512                                 # gather token-block size
N_GBLK = S // GBLK                         # 4


def _chunks(n, c=128):
    out = []
    o = 0
    while o < n:
        out.append((o, min(c, n - o)))
        o += c
    return out


def build_nc(mod=True, stream=True, mod_rep=1, stream_rep=1, sc_bufs=7):
    nc = bacc.Bacc("TRN2", target_bir_lowering=False, debug=False,
                   num_devices=N_CORES)

    dt = nc.dram_tensor
    scores = dt("scores", [SHARD_ROWS, S], F16, kind="ExternalInput")
    ids_row = dt("ids_row", [1, N_TOK], F16, kind="ExternalInput")
    ids_all = dt("ids_all", [128, N_TOK // 128], F32, kind="ExternalInput")
    iota_row = dt("iota_row", [1, CTX], F16, kind="ExternalInput")
    iota_cols = dt("iota_cols", [128, CTX_CHUNKS], F32, kind="ExternalInput")
    embT = dt("embT", [VD, CTX], F16, kind="ExternalInput")
    pW1 = dt("pW1", [VD, 2 * VD], F16, kind="ExternalInput")
    pb1 = dt("pb1", [128, 5], F32, kind="ExternalInput")
    pW2 = dt("pW2", [2 * VD, H], F16, kind="ExternalInput")
    pb2 = dt("pb2", [H, 1], F32, kind="ExternalInput")
    fW1 = dt("fW1", [H + 1, FFN], F16, kind="ExternalInput")
    fb1 = dt("fb1", [128, 4], F32, kind="ExternalInput")
    fW2 = dt("fW2", [FFN, FFN // 2], F16, kind="ExternalInput")
    fb2 = dt("fb2", [128, 2], F32, kind="ExternalInput")
    fW3 = dt("fW3", [FFN // 2, 1], F16, kind="ExternalInput")
    fb3 = dt("fb3", [1, 1], F32, kind="ExternalInput")
    lnvw = dt("lnvw", [128, 3], F32, kind="ExternalInput")   # VD cols chunked
    lnvb = dt("lnvb", [128, 3], F32, kind="ExternalInput")
    lncw = dt("lncw", [H + 1, 1], F32, kind="ExternalInput")
    lncb = dt("lncb", [H + 1, 1], F32, kind="ExternalInput")
    out = dt("out", [SHARD_ROWS, S], F16, kind="ExternalOutput")

    VD_CH = _chunks(VD)            # [(0,128),(128,128),(256,32)]
    VD2_CH = _chunks(2 * VD)       # 576 -> 5 chunks
    FFN_CH = _chunks(FFN)          # 512 -> 4
    FFNH_CH = _chunks(FFN // 2)    # 256 -> 2
    NSPLIT = [(0, 512), (512, 512)]   # vocab free-dim split

    with tile.TileContext(nc) as tc, ExitStack() as ctx:
        singles = ctx.enter_context(tc.tile_pool(name="singles", bufs=1))
        work = ctx.enter_context(tc.tile_pool(name="work", bufs=10))
        rows = ctx.enter_context(tc.tile_pool(name="rows", bufs=8))
        opool = ctx.enter_context(tc.tile_pool(name="opool", bufs=2))
        otpool = ctx.enter_context(tc.tile_pool(name="otpool", bufs=1))
        ps = ctx.enter_context(tc.tile_pool(name="ps", bufs=3, space="PSUM"))
        pcol = ctx.enter_context(tc.tile_pool(name="pcol", bufs=1, space="PSUM"))
        sc_pool = ctx.enter_context(tc.tile_pool(name="sc", bufs=sc_bufs))

        # ---------- constants / weights into SBUF ----------
        _uid = [0]

        def load(shape, src, dtype=F32):
            _uid[0] += 1
            t = singles.tile(shape, dtype, tag=f"s{_uid[0]}", name=f"s{_uid[0]}")
            nc.sync.dma_start(out=t[:], in_=src)
            return t

        def stile(shape, dtype=F32):
            _uid[0] += 1
            return singles.tile(shape, dtype, tag=f"s{_uid[0]}", name=f"s{_uid[0]}")

        iota_cols_sb = load([128, CTX_CHUNKS], iota_cols[:, :], F32)
        ids_all_sb = load([128, N_TOK // 128], ids_all[:, :], F32)
        ids_glob_b = stile([128, N_TOK], F16)
        nc.sync.dma_start(out=ids_glob_b[:],
                          in_=ids_row.ap().to_broadcast((128, N_TOK)))
        iota_b = stile([128, CTX], F16)
        nc.sync.dma_start(out=iota_b[:],
                          in_=iota_row.ap().to_broadcast((128, CTX)))

        embT_sb = []
        for o, sz in VD_CH:
            _t = work.tile([sz, CTX], F16, tag="work", name=f"embT{o}")
            nc.sync.dma_start(out=_t[:], in_=embT[o:o + sz, :])
            embT_sb.append(_t)
        pW1_sb = [load([sz, 2 * VD], pW1[o:o + sz, :], F16) for o, sz in VD_CH]
        pW2_sb = [load([sz, H], pW2[o:o + sz, :], F16) for o, sz in VD2_CH]
        fW1_sb = load([H + 1, FFN], fW1[:, :], F16)
        fW2_sb = [load([sz, FFN // 2], fW2[o:o + sz, :], F16) for o, sz in FFN_CH]
        fW3_sb = [load([sz, 1], fW3[o:o + sz, :], F16) for o, sz in FFNH_CH]
        pb1_sb = load([128, 5], pb1[:, :])
        pb2_sb = load([H, 1], pb2[:, :])
        fb1_sb = load([128, 4], fb1[:, :])
        fb2_sb = load([128, 2], fb2[:, :])
        fb3_sb = load([1, 1], fb3[:, :])
        lnvw_sb = load([128, 3], lnvw[:, :])
        lnvb_sb = load([128, 3], lnvb[:, :])
        lncw_sb = load([H + 1, 1], lncw[:, :])
        lncb_sb = load([H + 1, 1], lncb[:, :])

        ones_col_h = stile([128, 1], F16)
        nc.vector.memset(ones_col_h[:], 1.0)
        ones_row_h = stile([1, 128], F16)
        nc.vector.memset(ones_row_h[:], 1.0)
        identity = stile([128, 128], F32)
        make_identity(nc, identity[:])

        eps_sb = stile([128, 1], F32)
        nc.vector.memset(eps_sb[:], LN_EPS)
        modv16 = stile([128, CTX_CHUNKS], F16)
        # scales split into N_GBLK tiles so stream multiplies can start as
        # soon as their token block's gather lands.
        scales_sb = [stile([128, S_TILES // N_GBLK], F32) for _ in range(N_GBLK)]

        if not mod:
            for sct in scales_sb:
                nc.vector.memset(sct[:], 1.0)
        # the whole table pipeline runs in f16: mod feeds the output only
        # through 1 + eps*mod with eps=0.066, so f16's ~1e-3 error lands
        # ~1e-4 relative on the output (budget is 2e-2).
        nc._allow_low_precision_reason = "f16 mod table pipeline"
        for _mrep in range(mod_rep if mod else 0):
            # ---------- global token histogram (one-hot + PE accumulate) ---
            counts_ps = ps.tile([1, CTX], F32, tag="ps", name=f"crow{_mrep}")
            for t in range(N_TOK // 128):
                o_t = opool.tile([128, CTX], F16, tag="o", name=f"o{_mrep}_{t}")
                nc.vector.tensor_scalar(o_t[:], iota_b[:],
                                        ids_all_sb[:, t:t + 1], None,
                                        op0=ALU.is_equal)
                for n0, nsz in NSPLIT:
                    nc.tensor.matmul(counts_ps[:1, n0:n0 + nsz],
                                     ones_col_h[:, :], o_t[:, n0:n0 + nsz],
                                     start=(t == 0), stop=(t == N_TOK // 128 - 1))
            occ_sb = rows.tile([1, CTX], F16, tag="rows", name=f"occ{_mrep}")
            nc.scalar.activation(out=occ_sb[:], in_=counts_ps[:1, :],
                                 func=AF.Ln, bias=1.0)

            # ---------- LN_v of emb table, transposed layout ----------
            sum_ps = ps.tile([1, CTX], F32, tag="ps")
            sumsq_ps = ps.tile([1, CTX], F32, tag="ps")
            sq_t = []
            for k, (o, sz) in enumerate(VD_CH):
                sq = work.tile([sz, CTX], F16, tag="work")
                nc.vector.tensor_tensor(out=sq[:], in0=embT_sb[k][:],
                                        in1=embT_sb[k][:], op=ALU.mult)
                sq_t.append(sq)
            for n0, nsz in NSPLIT:
                for k, (o, sz) in enumerate(VD_CH):
                    nc.tensor.matmul(sum_ps[:1, n0:n0 + nsz],
                                     ones_col_h[:sz, :],
                                     embT_sb[k][:, n0:n0 + nsz],
                                     start=(k == 0), stop=(k == len(VD_CH) - 1))
                for k, (o, sz) in enumerate(VD_CH):
                    nc.tensor.matmul(sumsq_ps[:1, n0:n0 + nsz],
                                     ones_col_h[:sz, :],
                                     sq_t[k][:, n0:n0 + nsz],
                                     start=(k == 0), stop=(k == len(VD_CH) - 1))

            # stats rows: f16 straight from PSUM, f16 broadcast matmuls,
            # and the normalize reads the broadcast PSUM directly.
            m16 = rows.tile([1, CTX], F16, tag="rows")
            nc.vector.tensor_scalar_mul(m16[:], sum_ps[:1, :], 1.0 / VD)
            v16 = rows.tile([1, CTX], F16, tag="rows")
            nc.vector.tensor_scalar_mul(v16[:], sumsq_ps[:1, :], 1.0 / VD)
            msq16 = rows.tile([1, CTX], F16, tag="rows")
            nc.vector.tensor_tensor(out=msq16[:], in0=m16[:], in1=m16[:],
                                    op=ALU.mult)
            nc.vector.tensor_tensor(out=v16[:], in0=v16[:], in1=msq16[:],
                                    op=ALU.subtract)
            nc.scalar.activation(out=v16[:], in_=v16[:], func=AF.Sqrt,
                                 bias=eps_sb[:1, :])
            nc.vector.reciprocal(v16[:], v16[:])       # v16 = rstd row

            mean_ps = ps.tile([128, CTX], F32, tag="ps")
            rstd_ps = ps.tile([128, CTX], F32, tag="ps")
            for n0, nsz in NSPLIT:
                nc.tensor.matmul(mean_ps[:, n0:n0 + nsz], ones_row_h[:1, :],
                                 m16[:1, n0:n0 + nsz], start=True, stop=True)
                nc.tensor.matmul(rstd_ps[:, n0:n0 + nsz], ones_row_h[:1, :],
                                 v16[:1, n0:n0 + nsz], start=True, stop=True)

            # normalized emb table E'T (f16)
            e1t = []
            for k, (o, sz) in enumerate(VD_CH):
                e1 = work.tile([sz, CTX], F16, tag="work")
                nc.vector.tensor_tensor(out=e1[:], in0=embT_sb[k][:],
                                        in1=mean_ps[:sz, :], op=ALU.subtract)
                nc.vector.tensor_tensor(out=e1[:], in0=e1[:], in1=rstd_ps[:sz, :],
                                        op=ALU.mult)
                nc.vector.tensor_scalar(e1[:], e1[:], lnvw_sb[:sz, k:k + 1],
                                        lnvb_sb[:sz, k:k + 1],
                                        op0=ALU.mult, op1=ALU.add)
                e1t.append(e1)

            # ---------- layer A (gelu(E' @ pW1 + pb1)) fused with ----------
            # ---------- layer B (tanh(H1 @ pW2 + pb2)) accumulation ----------
            combT = work.tile([H + 1, CTX], F16, tag="work")
            val_ps = ps.tile([H, CTX], F32, tag="ps")
            for m, (mo, msz) in enumerate(VD2_CH):
                h1_ps = ps.tile([msz, CTX], F32, tag="ps")
                for n0, nsz in NSPLIT:
                    for k, (o, sz) in enumerate(VD_CH):
                        nc.tensor.matmul(h1_ps[:, n0:n0 + nsz],
                                         pW1_sb[k][:, mo:mo + msz],
                                         e1t[k][:, n0:n0 + nsz],
                                         start=(k == 0),
                                         stop=(k == len(VD_CH) - 1))
                h1 = work.tile([msz, CTX], F16, tag="work")
                nc.scalar.activation(out=h1[:], in_=h1_ps[:], func=AF.Gelu,
                                     bias=pb1_sb[:msz, m:m + 1])
                for n0, nsz in NSPLIT:
                    nc.tensor.matmul(val_ps[:, n0:n0 + nsz], pW2_sb[m][:, :],
                                     h1[:, n0:n0 + nsz],
                                     start=(m == 0), stop=(m == len(VD2_CH) - 1))

            # comb rows stored permuted as [valence(16 rows), occ] (host
            # permutes fW1 / ln_c to match) because SBUF partition-offset
            # writes must be 32-aligned -- the occ row lands in partition 16
            # via an SBUF->SBUF DMA instead.
            nc.sync.dma_start(out=combT[H:H + 1, :], in_=occ_sb[:1, :])
            nc.scalar.activation(out=combT[0:H, :], in_=val_ps[:], func=AF.Tanh,
                                 bias=pb2_sb[:H, 0:1])

            # ---------- LN_c over the 17 rows ----------
            sum17 = ps.tile([1, CTX], F32, tag="ps")
            sumsq17 = ps.tile([1, CTX], F32, tag="ps")
            sq17 = work.tile([H + 1, CTX], F16, tag="work")
            nc.vector.tensor_tensor(out=sq17[:], in0=combT[:], in1=combT[:],
                                    op=ALU.mult)
            for n0, nsz in NSPLIT:
                nc.tensor.matmul(sum17[:1, n0:n0 + nsz], ones_col_h[:H + 1, :],
                                 combT[:, n0:n0 + nsz], start=True, stop=True)
                nc.tensor.matmul(sumsq17[:1, n0:n0 + nsz], ones_col_h[:H + 1, :],
                                 sq17[:, n0:n0 + nsz], start=True, stop=True)
            m17 = rows.tile([1, CTX], F16, tag="rows")
            nc.vector.tensor_scalar_mul(m17[:], sum17[:1, :], 1.0 / (H + 1))
            v17 = rows.tile([1, CTX], F16, tag="rows")
            nc.vector.tensor_scalar_mul(v17[:], sumsq17[:1, :], 1.0 / (H + 1))
            msq17 = rows.tile([1, CTX], F16, tag="rows")
            nc.vector.tensor_tensor(out=msq17[:], in0=m17[:], in1=m17[:],
                                    op=ALU.mult)
            nc.vector.tensor_tensor(out=v17[:], in0=v17[:], in1=msq17[:],
                                    op=ALU.subtract)
            nc.scalar.activation(out=v17[:], in_=v17[:], func=AF.Sqrt,
                                 bias=eps_sb[:1, :])
            nc.vector.reciprocal(v17[:], v17[:])
            mean17_ps = ps.tile([H + 1, CTX], F32, tag="ps")
            rstd17_ps = ps.tile([H + 1, CTX], F32, tag="ps")
            for n0, nsz in NSPLIT:
                nc.tensor.matmul(mean17_ps[:, n0:n0 + nsz], ones_row_h[:1, :H + 1],
                                 m17[:1, n0:n0 + nsz], start=True, stop=True)
                nc.tensor.matmul(rstd17_ps[:, n0:n0 + nsz], ones_row_h[:1, :H + 1],
                                 v17[:1, n0:n0 + nsz], start=True, stop=True)
            comb2 = work.tile([H + 1, CTX], F16, tag="work")
            nc.vector.tensor_tensor(out=comb2[:], in0=combT[:], in1=mean17_ps[:],
                                    op=ALU.subtract)
            nc.vector.tensor_tensor(out=comb2[:], in0=comb2[:], in1=rstd17_ps[:],
                                    op=ALU.mult)
            nc.vector.tensor_scalar(comb2[:], comb2[:], lncw_sb[:, 0:1],
                                    lncb_sb[:, 0:1], op0=ALU.mult, op1=ALU.add)

            # ---------- layers D, E, F ----------
            h2t = []
            for m, (mo, msz) in enumerate(FFN_CH):
                h2_ps = ps.tile([msz, CTX], F32, tag="ps")
                for n0, nsz in NSPLIT:
                    nc.tensor.matmul(h2_ps[:, n0:n0 + nsz],
                                     fW1_sb[:, mo:mo + msz],
                                     comb2[:, n0:n0 + nsz], start=True, stop=True)
                h2 = work.tile([msz, CTX], F16, tag="work")
                nc.scalar.activation(out=h2[:], in_=h2_ps[:], func=AF.Gelu,
                                     bias=fb1_sb[:msz, m:m + 1])
                h2t.append(h2)
            h3t = []
            for m, (mo, msz) in enumerate(FFNH_CH):
                h3_ps = ps.tile([msz, CTX], F32, tag="ps")
                for n0, nsz in NSPLIT:
                    for k, (o, sz) in enumerate(FFN_CH):
                        nc.tensor.matmul(h3_ps[:, n0:n0 + nsz],
                                         fW2_sb[k][:, mo:mo + msz],
                                         h2t[k][:, n0:n0 + nsz],
                                         start=(k == 0),
                                         stop=(k == len(FFN_CH) - 1))
                h3 = work.tile([msz, CTX], F16, tag="work")
                nc.scalar.activation(out=h3[:], in_=h3_ps[:], func=AF.Gelu,
                                     bias=fb2_sb[:msz, m:m + 1])
                h3t.append(h3)
            mod_ps = ps.tile([1, CTX], F32, tag="ps")
            for n0, nsz in NSPLIT:
                for k, (o, sz) in enumerate(FFNH_CH):
                    nc.tensor.matmul(mod_ps[:1, n0:n0 + nsz], fW3_sb[k][:, :],
                                     h3t[k][:, n0:n0 + nsz],
                                     start=(k == 0), stop=(k == len(FFNH_CH) - 1))
            mod_row = rows.tile([1, CTX], F32, tag="rows")
            nc.scalar.activation(out=mod_row[:], in_=mod_ps[:1, :], func=AF.Tanh,
                                 bias=fb3_sb[:1, 0:1])

            # mod row -> per-chunk columns (PE transpose of [1,128] slices)
            modc_ps = pcol.tile([128, CTX_CHUNKS], F32, tag="pc")
            for c in range(CTX_CHUNKS):
                nc.tensor.transpose(modc_ps[:, c:c + 1],
                                    mod_row[:1, ts(c, 128)], identity[:1, :1])
            nc.vector.tensor_copy(modv16[:], modc_ps[:])   # cast f32 -> f16

            # ---------- gather per-token mod, one 512-token block at a ----
            # time so scales land incrementally (local batch occupies the
            # first S columns of ids_glob_b).
            for blk in range(N_GBLK):
                t0 = blk * GBLK
                ot_sb = []
                for c in range(CTX_CHUNKS):
                    _o = otpool.tile([128, GBLK], F16, tag=f"ot{c}",
                                     name=f"ot{_mrep}_{blk}_{c}")
                    nc.vector.tensor_scalar(
                        _o[:], ids_glob_b[:, t0:t0 + GBLK],
                        iota_cols_sb[:, c:c + 1], None, op0=ALU.is_equal)
                    ot_sb.append(_o)
                row_ps = ps.tile([1, GBLK], F32, tag="ps",
                                 name=f"grow{_mrep}_{blk}")
                for c in range(CTX_CHUNKS):
                    nc.tensor.matmul(row_ps[:1, :], modv16[:, c:c + 1],
                                     ot_sb[c][:, :],
                                     start=(c == 0), stop=(c == CTX_CHUNKS - 1))
                mrow_sb = rows.tile([1, GBLK], F32, tag="rows",
                                    name=f"mrow{_mrep}_{blk}")
                nc.vector.tensor_copy(mrow_sb[:], row_ps[:1, :])
                gath_ps = pcol.tile([128, GBLK // 128], F32, tag="pc2",
                                    name=f"gath{_mrep}_{blk}")
                for t in range(GBLK // 128):
                    nc.tensor.transpose(gath_ps[:, t:t + 1],
                                        mrow_sb[:1, ts(t, 128)], identity[:1, :1])
                nc.vector.tensor_scalar(scales_sb[blk][:], gath_ps[:],
                                        EPSILON, 1.0, op0=ALU.mult, op1=ALU.add)

        nc._allow_low_precision_reason = None
        if not stream:
            N_CH = 0
        else:
            N_CH = N_CHUNKS
        # ---------- the memory-bound scale of attention_scores ----------
        # f16 in / f16 out, multiplied in place (DVE f16 is far under the
        # DMA bound), single buffer per chunk cycling through sc_pool.
        for j in range(N_CH * stream_rep):
            j = j % N_CHUNKS
            r0 = j * ROWS_PER_CHUNK
            src = scores[r0:r0 + ROWS_PER_CHUNK, :].rearrange(
                "(t p) k -> p t k", p=128)
            dst = out[r0:r0 + ROWS_PER_CHUNK, :].rearrange(
                "(t p) k -> p t k", p=128)
            sc = sc_pool.tile([128, SUB_TILES, S], F16, tag="sc")
            nc.sync.dma_start(out=sc[:], in_=src)
            for t in range(SUB_TILES):
                qt = ((j * SUB_TILES) + t) % S_TILES
                blk, col = divmod(qt, S_TILES // N_GBLK)
                nc.vector.tensor_scalar_mul(sc[:, t, :], sc[:, t, :],
                                            scales_sb[blk][:, col:col + 1])
            nc.sync.dma_start(out=dst, in_=sc[:])

    nc.finalize()
    return nc


_NC = None


def _get_nc():
    global _NC
    if _NC is None:
        _NC = build_nc()
    return _NC


def _cols(v, ncols):
    out = np.zeros((128, ncols), np.float32)
    v = v.reshape(-1)
    for k, (o, sz) in enumerate(_chunks(len(v))):
        out[:sz, k] = v[o:o + sz]
    return out


def build_in_maps(inputs):
    scores = np.asarray(inputs["attention_scores"])
    ids = np.asarray(inputs["input_ids"]).astype(np.int64)

    iota_cols = np.ascontiguousarray(
        np.arange(CTX, dtype=np.float32).reshape(CTX_CHUNKS, 128).T)
    iota_row = np.arange(CTX, dtype=np.float16).reshape(1, CTX)
    ids_all = np.ascontiguousarray(
        ids.reshape(-1).reshape(N_TOK // 128, 128).T).astype(np.float32)

    f32 = lambda x: np.ascontiguousarray(np.asarray(x, dtype=np.float32))
    f16 = lambda x: np.ascontiguousarray(np.asarray(x, dtype=np.float32)
                                         .astype(np.float16))
    embT = f16(np.asarray(inputs["emb_W"]).T)
    lnvw = np.zeros((128, 3), np.float32)
    lnvb = np.zeros((128, 3), np.float32)
    wv = f32(inputs["ln_v_w"]).reshape(-1)
    bv = f32(inputs["ln_v_b"]).reshape(-1)
    for k, (o, sz) in enumerate(_chunks(VD)):
        lnvw[:sz, k] = wv[o:o + sz]
        lnvb[:sz, k] = bv[o:o + sz]

    common = {
        "iota_cols": iota_cols,
        "iota_row": iota_row,
        "ids_all": ids_all,
        "embT": embT,
        "pW1": f16(inputs["pW1"]), "pb1": _cols(f32(inputs["pb1"]), 5),
        "pW2": f16(inputs["pW2"]), "pb2": f32(inputs["pb2"]).reshape(-1, 1),
        "fW1": f16(np.roll(f32(inputs["fW1"]), -1, axis=0)),
        "fb1": _cols(f32(inputs["fb1"]), 4),
        "fW2": f16(inputs["fW2"]), "fb2": _cols(f32(inputs["fb2"]), 2),
        "fW3": f16(inputs["fW3"]), "fb3": f32(inputs["fb3"]).reshape(1, -1),
        "lnvw": lnvw, "lnvb": lnvb,
        "lncw": np.roll(f32(inputs["ln_c_w"]), -1).reshape(-1, 1),
        "lncb": np.roll(f32(inputs["ln_c_b"]), -1).reshape(-1, 1),
    }

    scores_flat = scores.reshape(B * H, S, S)
    in_maps = []
    for i in range(N_CORES):
        b = i // (N_CORES // B)
        shard = np.ascontiguousarray(
            scores_flat[i * HEADS_PER_CORE:(i + 1) * HEADS_PER_CORE]
        ).reshape(SHARD_ROWS, S).astype(np.float16)
        m = dict(common)
        m["scores"] = shard
        # local batch first so gather blocks index columns 0..S-1 directly
        m["ids_row"] = np.concatenate([ids[b], ids[1 - b]]).astype(
            np.float16).reshape(1, N_TOK)
        in_maps.append(m)
    return in_maps


def assemble_output(core_outs):
    shards = [core_outs[i]["out"] for i in range(N_CORES)]
    return np.concatenate(shards, axis=0).reshape(B, H, S, S).astype(np.float32)


def _run(inputs, **spmd_kwargs):
    in_maps = build_in_maps(inputs)
    nc = _get_nc()
    res = run_bass_kernel_spmd(nc, in_maps, core_ids=list(range(N_CORES)),
                               **spmd_kwargs)
    out = assemble_output(res.results)
    return out, res


def kernel(**inputs) -> np.ndarray:
    return _run(inputs)[0]


if __name__ == "__main__":
    rng = np.random.default_rng(0)
    inputs = {
        "attention_scores": rng.standard_normal((B, H, S, S), dtype=np.float32),
        "input_ids": rng.integers(0, CTX, size=(B, S)),
        "emb_W": rng.standard_normal((CTX, VD), dtype=np.float32) * 0.05,
        "ln_v_w": np.ones(VD, np.float32), "ln_v_b": np.zeros(VD, np.float32),
        "pW1": rng.standard_normal((VD, 2 * VD), dtype=np.float32) * 0.05,
        "pb1": rng.standard_normal(2 * VD, dtype=np.float32) * 0.05,
        "pW2": rng.standard_normal((2 * VD, H), dtype=np.float32) * 0.04,
        "pb2": rng.standard_normal(H, dtype=np.float32) * 0.04,
        "ln_c_w": np.ones(H + 1, np.float32), "ln_c_b": np.zeros(H + 1, np.float32),
        "fW1": rng.standard_normal((H + 1, FFN), dtype=np.float32) * 0.2,
        "fb1": rng.standard_normal(FFN, dtype=np.float32) * 0.2,
        "fW2": rng.standard_normal((FFN, FFN // 2), dtype=np.float32) * 0.04,
        "fb2": rng.standard_normal(FFN // 2, dtype=np.float32) * 0.04,
        "fW3": rng.standard_normal((FFN // 2, 1), dtype=np.float32) * 0.06,
        "fb3": rng.standard_normal(1, dtype=np.float32) * 0.06,
    }
    out = kernel(**inputs)
    print("kernel output", out.shape, out.dtype, float(np.abs(out).mean()))


# revision 51
# speedup vs baseline: 16.8261x; 16.8261x over previous
"""Trainium2 Bass kernel for nn_MemoryWeightedAttention.

out[b,h,q,k] = attention_scores[b,h,q,k] * (1 + 0.066 * mod[b,q])

where mod[b,q] is a small LN/MLP pipeline applied to
(log1p(global_count[id]), tanh-MLP(LN(emb[id]))) -- i.e. mod depends ONLY on
the token id at (b,q) plus a global histogram of input_ids.  So we compute the
full pipeline once over the 1024-entry vocabulary table (in transposed layout:
features on partitions, vocab on the free dim -> no transposes between
layers), then gather per-token scales with one-hot matmuls, and stream the
512MB attention_scores tensor through a per-partition scalar multiply.

Perf structure: streaming is f16 in / f16 out (rel-err budget is 2e-2; f16
costs ~2.4e-4), which halves HBM traffic vs f32; loads go on the sync HWDGE
ring and stores on the scalar HWDGE ring so per-DMA completion stalls on the
two rings overlap; the table pipeline runs in f16 on PE (4x faster matmuls),
small params are packed into a handful of DMAs, activation-table reloads are
minimized (copy/square live in every ACT table), and the gather emits scales
in 4 token blocks so the first stream multiplies start before the whole
gather finishes.  The streaming chunks (8 x 2 MiB in flight) prefetch while
the replicated mod pipeline computes.

Sharding: 8 cores, core i handles batch i//4, heads 4*(i%4) .. 4*(i%4)+4
(a contiguous [8192, 2048] row-slice of the flattened [65536, 2048] scores).
The tiny table pipeline is replicated on every core (no collectives).
"""

import sys

for _p in ("/opt/trn_rl_repo",):
    if _p not in sys.path:
        sys.path.insert(0, _p)

from contextlib import ExitStack

import numpy as np

import concourse.bacc as bacc
import concourse.tile as tile
from concourse import mybir
from concourse.bass import ts
from concourse.bass_utils import run_bass_kernel_spmd
from concourse.masks import make_identity

F32 = mybir.dt.float32
F16 = mybir.dt.float16
AF = mybir.ActivationFunctionType
ALU = mybir.AluOpType
AX = mybir.AxisListType

B, H, S = 2, 16, 2048
CTX = 1024
VD = 288          # valence dim
FFN = 512
EPSILON = 0.066
LN_EPS = 1e-5

N_CORES = 8
HEADS_PER_CORE = H * B // N_CORES          # 4
SHARD_ROWS = HEADS_PER_CORE * S            # 8192
N_TOK = B * S                              # 4096 (global, for counts)
S_TILES = S // 128                         # 16 q tiles per core's batch
CTX_CHUNKS = CTX // 128                    # 8
ROWS_PER_CHUNK = 512                       # score rows per DMA chunk
N_CHUNKS = SHARD_ROWS // ROWS_PER_CHUNK    # 16
SUB_TILES = ROWS_PER_CHUNK // 128          # 4
GBLK = 512                                 # gather token-block size
N_GBLK = S // GBLK                         # 4


def _chunks(n, c=128):
    out = []
    o = 0
    while o < n:
        out.append((o, min(c, n - o)))
        o += c
    return out


WP32 = dict(iota_cols=(0, 8), ids_all=(8, 32), pb1=(40, 5), fb1=(45, 4),
            fb2=(49, 2), lnvw=(51, 3), lnvb=(54, 3), pb2=(57, 1),
            lncw=(58, 1), lncb=(59, 1), fb3=(60, 1))
WP32_N = 61
WP16_N = 2 * VD // 128 * 16 + 16 + 2        # pW2 (5 chunks x16) + fW3 (2)


def build_nc(mod=True, stream=True, mod_rep=1, stream_rep=1, sc_bufs=8,
             rows_per_chunk=ROWS_PER_CHUNK, store_engine="scalar",
             inplace=True):
    nc = bacc.Bacc("TRN2", target_bir_lowering=False, debug=False,
                   num_devices=N_CORES)

    dt = nc.dram_tensor
    scores = dt("scores", [SHARD_ROWS, S], F16, kind="ExternalInput")
    ids_row = dt("ids_row", [1, N_TOK], F16, kind="ExternalInput")
    iota_row = dt("iota_row", [1, CTX], F16, kind="ExternalInput")
    # packed small params: one DMA each instead of ~25 serial issues
    wp32 = dt("wp32", [128, WP32_N], F32, kind="ExternalInput")
    wp16 = dt("wp16", [128, WP16_N], F16, kind="ExternalInput")
    embT = dt("embT", [128, 3 * CTX], F16, kind="ExternalInput")   # 3 row-chunks packed
    pW1 = dt("pW1", [128, 3 * 2 * VD], F16, kind="ExternalInput")  # 3 row-chunks packed
    fW1 = dt("fW1", [H + 1, FFN], F16, kind="ExternalInput")
    fW2 = dt("fW2", [128, 2 * FFN], F16, kind="ExternalInput")     # 4 row-chunks packed
    out = dt("out", [SHARD_ROWS, S], F16, kind="ExternalOutput")

    VD_CH = _chunks(VD)            # [(0,128),(128,128),(256,32)]
    VD2_CH = _chunks(2 * VD)       # 576 -> 5 chunks
    FFN_CH = _chunks(FFN)          # 512 -> 4
    FFNH_CH = _chunks(FFN // 2)    # 256 -> 2
    NSPLIT = [(0, 512), (512, 512)]   # vocab free-dim split

    with tile.TileContext(nc) as tc, ExitStack() as ctx:
        singles = ctx.enter_context(tc.tile_pool(name="singles", bufs=1))
        work = ctx.enter_context(tc.tile_pool(name="work", bufs=10))
        rows = ctx.enter_context(tc.tile_pool(name="rows", bufs=6))
        opool = ctx.enter_context(tc.tile_pool(name="opool", bufs=2))
        otpool = ctx.enter_context(tc.tile_pool(name="otpool", bufs=1))
        ps = ctx.enter_context(tc.tile_pool(name="ps", bufs=3, space="PSUM"))
        pcol = ctx.enter_context(tc.tile_pool(name="pcol", bufs=1, space="PSUM"))
        sc_pool = ctx.enter_context(tc.tile_pool(name="sc", bufs=sc_bufs))

        # ---------- constants / weights into SBUF ----------
        _uid = [0]

        def load(shape, src, dtype=F32):
            _uid[0] += 1
            t = singles.tile(shape, dtype, tag=f"s{_uid[0]}", name=f"s{_uid[0]}")
            nc.sync.dma_start(out=t[:], in_=src)
            return t

        def stile(shape, dtype=F32):
            _uid[0] += 1
            return singles.tile(shape, dtype, tag=f"s{_uid[0]}", name=f"s{_uid[0]}")

        # embT first: it heads the mod critical path
        embp = load([128, 3 * CTX], embT[:, :], F16)
        embT_sb = [embp[:sz, k * CTX:(k + 1) * CTX] for k, (o, sz) in enumerate(VD_CH)]
        pW1p = load([128, 3 * 2 * VD], pW1[:, :], F16)
        pW1_sb = [pW1p[:sz, k * 2 * VD:(k + 1) * 2 * VD]
                  for k, (o, sz) in enumerate(VD_CH)]
        wp32_sb = load([128, WP32_N], wp32[:, :], F32)
        wp16_sb = load([128, WP16_N], wp16[:, :], F16)
        fW1_sb = load([H + 1, FFN], fW1[:, :], F16)
        fW2p = load([128, 2 * FFN], fW2[:, :], F16)
        fW2_sb = [fW2p[:, k * (FFN // 2):(k + 1) * (FFN // 2)]
                  for k, (o, sz) in enumerate(FFN_CH)]

        def w32(key):
            o, n = WP32[key]
            return wp32_sb[:, o:o + n]

        iota_cols_sb = w32("iota_cols")
        ids_all_sb = w32("ids_all")
        pb1_sb, fb1_sb, fb2_sb = w32("pb1"), w32("fb1"), w32("fb2")
        lnvw_sb, lnvb_sb = w32("lnvw"), w32("lnvb")
        pb2_sb, lncw_sb, lncb_sb = w32("pb2"), w32("lncw"), w32("lncb")
        fb3_sb = w32("fb3")
        pW2_sb = [wp16_sb[:sz, m * H:(m + 1) * H] for m, (o, sz) in enumerate(VD2_CH)]
        fW3_sb = [wp16_sb[:sz, 5 * H + k:5 * H + k + 1]
                  for k, (o, sz) in enumerate(FFNH_CH)]

        ids_glob_b = stile([128, N_TOK], F16)
        nc.sync.dma_start(out=ids_glob_b[:],
                          in_=ids_row.ap().to_broadcast((128, N_TOK)))
        iota_b = stile([128, CTX], F16)
        nc.sync.dma_start(out=iota_b[:],
                          in_=iota_row.ap().to_broadcast((128, CTX)))

        ones_col_h = stile([128, 1], F16)
        nc.vector.memset(ones_col_h[:], 1.0)
        ones_row_h = stile([1, 128], F16)
        nc.vector.memset(ones_row_h[:], 1.0)
        identity = stile([128, 128], F32)
        make_identity(nc, identity[:])

        eps_sb = stile([128, 1], F32)
        nc.vector.memset(eps_sb[:], LN_EPS)
        modv16 = stile([128, CTX_CHUNKS], F16)
        # scales split into N_GBLK tiles so stream multiplies can start as
        # soon as their token block's gather lands.
        scales_sb = [stile([128, S_TILES // N_GBLK], F32) for _ in range(N_GBLK)]

        if not mod:
            for sct in scales_sb:
                nc.vector.memset(sct[:], 1.0)
        # the whole table pipeline runs in f16: mod feeds the output only
        # through 1 + eps*mod with eps=0.066, so f16's ~1e-3 error lands
        # ~1e-4 relative on the output (budget is 2e-2).
        nc._allow_low_precision_reason = "f16 mod table pipeline"
        for _mrep in range(mod_rep if mod else 0):
            # ---------- LN_v of emb table, transposed layout ----------
            sum_ps = ps.tile([1, CTX], F32, tag="ps")
            sumsq_ps = ps.tile([1, CTX], F32, tag="ps")
            sq_t = []
            for k, (o, sz) in enumerate(VD_CH):
                sq = work.tile([sz, CTX], F16, tag="work")
                nc.vector.tensor_tensor(out=sq[:], in0=embT_sb[k][:],
                                        in1=embT_sb[k][:], op=ALU.mult)
                sq_t.append(sq)
            for n0, nsz in NSPLIT:
                for k, (o, sz) in enumerate(VD_CH):
                    nc.tensor.matmul(sum_ps[:1, n0:n0 + nsz],
                                     ones_col_h[:sz, :],
                                     embT_sb[k][:, n0:n0 + nsz],
                                     start=(k == 0), stop=(k == len(VD_CH) - 1))
                for k, (o, sz) in enumerate(VD_CH):
                    nc.tensor.matmul(sumsq_ps[:1, n0:n0 + nsz],
                                     ones_col_h[:sz, :],
                                     sq_t[k][:, n0:n0 + nsz],
                                     start=(k == 0), stop=(k == len(VD_CH) - 1))

            # stats rows on ACT (copy/square/rsqrt share one act table):
            # m = sum/VD, msq = (sum/VD)^2, v = sumsq/VD - msq, rstd =
            # rsqrt(v + eps).
            m16 = rows.tile([1, CTX], F16, tag="rows")
            nc.scalar.activation(out=m16[:], in_=sum_ps[:1, :], func=AF.Copy,
                                 scale=1.0 / VD)
            msq16 = rows.tile([1, CTX], F16, tag="rows")
            nc.scalar.activation(out=msq16[:], in_=sum_ps[:1, :], func=AF.Square,
                                 scale=1.0 / VD)
            v16 = rows.tile([1, CTX], F16, tag="rows")
            nc.vector.scalar_tensor_tensor(out=v16[:], in0=sumsq_ps[:1, :],
                                           scalar=1.0 / VD, in1=msq16[:],
                                           op0=ALU.mult, op1=ALU.subtract)
            r16 = rows.tile([1, CTX], F16, tag="rows")
            nc.scalar.activation(out=r16[:], in_=v16[:], func=AF.Sqrt,
                                 bias=eps_sb[:1, :])
            nc.vector.reciprocal(r16[:], r16[:])

            mean_ps = ps.tile([128, CTX], F32, tag="ps")
            rstd_ps = ps.tile([128, CTX], F32, tag="ps")
            for n0, nsz in NSPLIT:
                nc.tensor.matmul(mean_ps[:, n0:n0 + nsz], ones_row_h[:1, :],
                                 m16[:1, n0:n0 + nsz], start=True, stop=True)
                nc.tensor.matmul(rstd_ps[:, n0:n0 + nsz], ones_row_h[:1, :],
                                 r16[:1, n0:n0 + nsz], start=True, stop=True)

            # ---------- global token histogram (one-hot + PE accumulate) ---
            # issued after the LN_v stats so the DVE one-hot builds don't
            # delay the critical chain; occ is only needed by LN_c.
            counts_ps = ps.tile([1, CTX], F32, tag="ps", name=f"crow{_mrep}")
            for t in range(N_TOK // 128):
                o_t = opool.tile([128, CTX], F16, tag="o", name=f"o{_mrep}_{t}")
                nc.vector.tensor_scalar(o_t[:], iota_b[:],
                                        ids_all_sb[:, t:t + 1], None,
                                        op0=ALU.is_equal)
                for n0, nsz in NSPLIT:
                    nc.tensor.matmul(counts_ps[:1, n0:n0 + nsz],
                                     ones_col_h[:, :], o_t[:, n0:n0 + nsz],
                                     start=(t == 0), stop=(t == N_TOK // 128 - 1))
            occ_sb = rows.tile([1, CTX], F16, tag="rows", name=f"occ{_mrep}")
            nc.scalar.activation(out=occ_sb[:], in_=counts_ps[:1, :],
                                 func=AF.Ln, bias=1.0)

            # normalized emb table E'T (f16)
            e1t = []
            for k, (o, sz) in enumerate(VD_CH):
                e1 = work.tile([sz, CTX], F16, tag="work")
                nc.vector.tensor_tensor(out=e1[:], in0=embT_sb[k][:],
                                        in1=mean_ps[:sz, :], op=ALU.subtract)
                nc.vector.tensor_tensor(out=e1[:], in0=e1[:], in1=rstd_ps[:sz, :],
                                        op=ALU.mult)
                nc.vector.tensor_scalar(e1[:], e1[:], lnvw_sb[:sz, k:k + 1],
                                        lnvb_sb[:sz, k:k + 1],
                                        op0=ALU.mult, op1=ALU.add)
                e1t.append(e1)

            # ---------- layer A (gelu(E' @ pW1 + pb1)) fused with ----------
            # ---------- layer B (tanh(H1 @ pW2 + pb2)) accumulation ----------
            combT = work.tile([H + 1, CTX], F16, tag="work")
            val_ps = ps.tile([H, CTX], F32, tag="ps")
            for m, (mo, msz) in enumerate(VD2_CH):
                h1_ps = ps.tile([msz, CTX], F32, tag="ps")
                for n0, nsz in NSPLIT:
                    for k, (o, sz) in enumerate(VD_CH):
                        nc.tensor.matmul(h1_ps[:, n0:n0 + nsz],
                                         pW1_sb[k][:, mo:mo + msz],
                                         e1t[k][:, n0:n0 + nsz],
                                         start=(k == 0),
                                         stop=(k == len(VD_CH) - 1))
                h1 = work.tile([msz, CTX], F16, tag="work")
                nc.scalar.activation(out=h1[:], in_=h1_ps[:], func=AF.Gelu,
                                     bias=pb1_sb[:msz, m:m + 1])
                for n0, nsz in NSPLIT:
                    nc.tensor.matmul(val_ps[:, n0:n0 + nsz], pW2_sb[m][:, :],
                                     h1[:, n0:n0 + nsz],
                                     start=(m == 0), stop=(m == len(VD2_CH) - 1))

            # comb rows stored permuted as [valence(16 rows), occ] (host
            # permutes fW1 / ln_c to match) because SBUF partition-offset
            # writes must be 32-aligned -- the occ row lands in partition 16
            # via an SBUF->SBUF DMA instead.
            nc.sync.dma_start(out=combT[H:H + 1, :], in_=occ_sb[:1, :])
            nc.scalar.activation(out=combT[0:H, :], in_=val_ps[:], func=AF.Tanh,
                                 bias=pb2_sb[:H, 0:1])

            # ---------- LN_c over the 17 rows ----------
            sum17 = ps.tile([1, CTX], F32, tag="ps")
            sumsq17 = ps.tile([1, CTX], F32, tag="ps")
            sq17 = work.tile([H + 1, CTX], F16, tag="work")
            nc.vector.tensor_tensor(out=sq17[:], in0=combT[:], in1=combT[:],
                                    op=ALU.mult)
            for n0, nsz in NSPLIT:
                nc.tensor.matmul(sum17[:1, n0:n0 + nsz], ones_col_h[:H + 1, :],
                                 combT[:, n0:n0 + nsz], start=True, stop=True)
                nc.tensor.matmul(sumsq17[:1, n0:n0 + nsz], ones_col_h[:H + 1, :],
                                 sq17[:, n0:n0 + nsz], start=True, stop=True)
            m17 = rows.tile([1, CTX], F16, tag="rows")
            nc.scalar.activation(out=m17[:], in_=sum17[:1, :], func=AF.Copy,
                                 scale=1.0 / (H + 1))
            msq17 = rows.tile([1, CTX], F16, tag="rows")
            nc.scalar.activation(out=msq17[:], in_=sum17[:1, :], func=AF.Square,
                                 scale=1.0 / (H + 1))
            v17 = rows.tile([1, CTX], F16, tag="rows")
            nc.vector.scalar_tensor_tensor(out=v17[:], in0=sumsq17[:1, :],
                                           scalar=1.0 / (H + 1), in1=msq17[:],
                                           op0=ALU.mult, op1=ALU.subtract)
            r17 = rows.tile([1, CTX], F16, tag="rows")
            nc.scalar.activation(out=r17[:], in_=v17[:], func=AF.Sqrt,
                                 bias=eps_sb[:1, :])
            nc.vector.reciprocal(r17[:], r17[:])
            mean17_ps = ps.tile([H + 1, CTX], F32, tag="ps")
            rstd17_ps = ps.tile([H + 1, CTX], F32, tag="ps")
            for n0, nsz in NSPLIT:
                nc.tensor.matmul(mean17_ps[:, n0:n0 + nsz], ones_row_h[:1, :H + 1],
                                 m17[:1, n0:n0 + nsz], start=True, stop=True)
                nc.tensor.matmul(rstd17_ps[:, n0:n0 + nsz], ones_row_h[:1, :H + 1],
                                 r17[:1, n0:n0 + nsz], start=True, stop=True)
            comb2 = work.tile([H + 1, CTX], F16, tag="work")
            nc.vector.tensor_tensor(out=comb2[:], in0=combT[:], in1=mean17_ps[:],
                                    op=ALU.subtract)
            nc.vector.tensor_tensor(out=comb2[:], in0=comb2[:], in1=rstd17_ps[:],
                                    op=ALU.mult)
            nc.vector.tensor_scalar(comb2[:], comb2[:], lncw_sb[:H + 1, 0:1],
                                    lncb_sb[:H + 1, 0:1],
                                    op0=ALU.mult, op1=ALU.add)

            # ---------- layers D, E, F ----------
            h2t = []
            for m, (mo, msz) in enumerate(FFN_CH):
                h2_ps = ps.tile([msz, CTX], F32, tag="ps")
                for n0, nsz in NSPLIT:
                    nc.tensor.matmul(h2_ps[:, n0:n0 + nsz],
                                     fW1_sb[:, mo:mo + msz],
                                     comb2[:, n0:n0 + nsz], start=True, stop=True)
                h2 = work.tile([msz, CTX], F16, tag="work")
                nc.scalar.activation(out=h2[:], in_=h2_ps[:], func=AF.Gelu,
                                     bias=fb1_sb[:msz, m:m + 1])
                h2t.append(h2)
            h3t = []
            for m, (mo, msz) in enumerate(FFNH_CH):
                h3_ps = ps.tile([msz, CTX], F32, tag="ps")
                for n0, nsz in NSPLIT:
                    for k, (o, sz) in enumerate(FFN_CH):
                        nc.tensor.matmul(h3_ps[:, n0:n0 + nsz],
                                         fW2_sb[k][:, mo:mo + msz],
                                         h2t[k][:, n0:n0 + nsz],
                                         start=(k == 0),
                                         stop=(k == len(FFN_CH) - 1))
                h3 = work.tile([msz, CTX], F16, tag="work")
                nc.scalar.activation(out=h3[:], in_=h3_ps[:], func=AF.Gelu,
                                     bias=fb2_sb[:msz, m:m + 1])
                h3t.append(h3)
            mod_ps = ps.tile([1, CTX], F32, tag="ps")
            for n0, nsz in NSPLIT:
                for k, (o, sz) in enumerate(FFNH_CH):
                    nc.tensor.matmul(mod_ps[:1, n0:n0 + nsz], fW3_sb[k][:, :],
                                     h3t[k][:, n0:n0 + nsz],
                                     start=(k == 0), stop=(k == len(FFNH_CH) - 1))
            mod_row = rows.tile([1, CTX], F32, tag="rows")
            nc.scalar.activation(out=mod_row[:], in_=mod_ps[:1, :], func=AF.Tanh,
                                 bias=fb3_sb[:1, 0:1])

            # mod row -> per-chunk columns (PE transpose of [1,128] slices)
            modc_ps = pcol.tile([128, CTX_CHUNKS], F32, tag="pc")
            for c in range(CTX_CHUNKS):
                nc.tensor.transpose(modc_ps[:, c:c + 1],
                                    mod_row[:1, ts(c, 128)], identity[:1, :1])
            nc.vector.tensor_copy(modv16[:], modc_ps[:])   # cast f32 -> f16

            # ---------- gather per-token mod, one 512-token block at a ----
            # time so scales land incrementally (local batch occupies the
            # first S columns of ids_glob_b).
            for blk in range(N_GBLK):
                t0 = blk * GBLK
                ot_sb = []
                for c in range(CTX_CHUNKS):
                    _o = otpool.tile([128, GBLK], F16, tag=f"ot{c}",
                                     name=f"ot{_mrep}_{blk}_{c}")
                    nc.vector.tensor_scalar(
                        _o[:], ids_glob_b[:, t0:t0 + GBLK],
                        iota_cols_sb[:, c:c + 1], None, op0=ALU.is_equal)
                    ot_sb.append(_o)
                row_ps = ps.tile([1, GBLK], F32, tag="ps",
                                 name=f"grow{_mrep}_{blk}")
                for c in range(CTX_CHUNKS):
                    nc.tensor.matmul(row_ps[:1, :], modv16[:, c:c + 1],
                                     ot_sb[c][:, :],
                                     start=(c == 0), stop=(c == CTX_CHUNKS - 1))
                mrow_sb = rows.tile([1, GBLK], F32, tag="rows",
                                    name=f"mrow{_mrep}_{blk}")
                nc.vector.tensor_copy(mrow_sb[:], row_ps[:1, :])
                gath_ps = pcol.tile([128, GBLK // 128], F32, tag="pc2",
                                    name=f"gath{_mrep}_{blk}")
                for t in range(GBLK // 128):
                    nc.tensor.transpose(gath_ps[:, t:t + 1],
                                        mrow_sb[:1, ts(t, 128)],
                                        identity[:1, :1])
                nc.vector.tensor_scalar(scales_sb[blk][:], gath_ps[:],
                                        EPSILON, 1.0, op0=ALU.mult, op1=ALU.add)

        nc._allow_low_precision_reason = None
        n_chunks = SHARD_ROWS // rows_per_chunk
        sub_tiles = rows_per_chunk // 128
        N_CH = n_chunks if stream else 0
        # ---------- the memory-bound scale of attention_scores ----------
        # f16 in / f16 out, multiplied in place (DVE f16 is far under the
        # DMA bound), single buffer per chunk cycling through sc_pool.
        store_eng = nc.sync if store_engine == "sync" else nc.scalar
        for j in range(N_CH * stream_rep):
            j = j % n_chunks
            r0 = j * rows_per_chunk
            src = scores[r0:r0 + rows_per_chunk, :].rearrange(
                "(t p) k -> p t k", p=128)
            dst = out[r0:r0 + rows_per_chunk, :].rearrange(
                "(t p) k -> p t k", p=128)
            sc = sc_pool.tile([128, sub_tiles, S], F16, tag="sc")
            nc.sync.dma_start(out=sc[:], in_=src)
            ot = sc if inplace else sc_pool.tile([128, sub_tiles, S], F16,
                                                 tag="so")
            for t in range(sub_tiles):
                qt = ((j * sub_tiles) + t) % S_TILES
                blk, col = divmod(qt, S_TILES // N_GBLK)
                nc.vector.tensor_scalar_mul(ot[:, t, :], sc[:, t, :],
                                            scales_sb[blk][:, col:col + 1])
            store_eng.dma_start(out=dst, in_=ot[:])

    nc.finalize()
    return nc


_NC = None


def _get_nc():
    global _NC
    if _NC is None:
        _NC = build_nc()
    return _NC


def _cols(v, ncols):
    out = np.zeros((128, ncols), np.float32)
    v = v.reshape(-1)
    for k, (o, sz) in enumerate(_chunks(len(v))):
        out[:sz, k] = v[o:o + sz]
    return out


def build_in_maps(inputs):
    scores = np.asarray(inputs["attention_scores"])
    ids = np.asarray(inputs["input_ids"]).astype(np.int64)

    f32 = lambda x: np.ascontiguousarray(np.asarray(x, dtype=np.float32))
    f16 = lambda x: np.ascontiguousarray(np.asarray(x, dtype=np.float32)
                                         .astype(np.float16))

    def packcols(dst, key, arr):
        o, n = WP32[key]
        a = np.asarray(arr, dtype=np.float32)
        if a.ndim == 1:
            a = a.reshape(-1, 1) if n == 1 else a
        if a.ndim == 1:
            for k, (co, sz) in enumerate(_chunks(len(a))):
                dst[:sz, o + k] = a[co:co + sz]
        else:
            dst[:a.shape[0], o:o + a.shape[1]] = a

    wp32 = np.zeros((128, WP32_N), np.float32)
    packcols(wp32, "iota_cols",
             np.arange(CTX, dtype=np.float32).reshape(CTX_CHUNKS, 128).T)
    packcols(wp32, "ids_all",
             np.ascontiguousarray(ids.reshape(-1).reshape(N_TOK // 128, 128).T))
    wp32_1d = dict(pb1=f32(inputs["pb1"]), fb1=f32(inputs["fb1"]),
                   fb2=f32(inputs["fb2"]), lnvw=f32(inputs["ln_v_w"]),
                   lnvb=f32(inputs["ln_v_b"]))
    for k, v in wp32_1d.items():
        o, n = WP32[k]
        for c, (co, sz) in enumerate(_chunks(len(v))):
            wp32[:sz, o + c] = v[co:co + sz]
    packcols(wp32, "pb2", f32(inputs["pb2"]).reshape(-1, 1))
    packcols(wp32, "lncw", np.roll(f32(inputs["ln_c_w"]), -1).reshape(-1, 1))
    packcols(wp32, "lncb", np.roll(f32(inputs["ln_c_b"]), -1).reshape(-1, 1))
    packcols(wp32, "fb3", f32(inputs["fb3"]).reshape(1, 1))

    wp16 = np.zeros((128, WP16_N), np.float16)
    pW2 = f16(inputs["pW2"])
    for m, (o, sz) in enumerate(_chunks(2 * VD)):
        wp16[:sz, m * H:(m + 1) * H] = pW2[o:o + sz, :]
    fW3 = f16(inputs["fW3"])
    for k, (o, sz) in enumerate(_chunks(FFN // 2)):
        wp16[:sz, 5 * H + k] = fW3[o:o + sz, 0]

    def packrows(arr, chunk_rows=128):
        a = np.asarray(arr)
        ncol = a.shape[1]
        nch = (a.shape[0] + chunk_rows - 1) // chunk_rows
        out_a = np.zeros((128, nch * ncol), a.dtype)
        for k in range(nch):
            rows_k = a[k * chunk_rows:(k + 1) * chunk_rows]
            out_a[:rows_k.shape[0], k * ncol:(k + 1) * ncol] = rows_k
        return out_a

    iota_row = np.arange(CTX, dtype=np.float16).reshape(1, CTX)

    common = {
        "iota_row": iota_row,
        "wp32": wp32,
        "wp16": wp16,
        "embT": packrows(f16(np.asarray(inputs["emb_W"]).T)),
        "pW1": packrows(f16(inputs["pW1"])),
        "fW1": f16(np.roll(f32(inputs["fW1"]), -1, axis=0)),
        "fW2": packrows(f16(inputs["fW2"])),
    }

    scores_flat = scores.reshape(B * H, S, S)
    in_maps = []
    for i in range(N_CORES):
        b = i // (N_CORES // B)
        shard = np.ascontiguousarray(
            scores_flat[i * HEADS_PER_CORE:(i + 1) * HEADS_PER_CORE]
        ).reshape(SHARD_ROWS, S).astype(np.float16)
        m = dict(common)
        m["scores"] = shard
        # local batch first so gather blocks index columns 0..S-1 directly
        m["ids_row"] = np.concatenate([ids[b], ids[1 - b]]).astype(
            np.float16).reshape(1, N_TOK)
        in_maps.append(m)
    return in_maps


def assemble_output(core_outs):
    shards = [core_outs[i]["out"] for i in range(N_CORES)]
    return np.concatenate(shards, axis=0).reshape(B, H, S, S).astype(np.float32)


def _run(inputs, **spmd_kwargs):
    in_maps = build_in_maps(inputs)
    nc = _get_nc()
    res = run_bass_kernel_spmd(nc, in_maps, core_ids=list(range(N_CORES)),
                               **spmd_kwargs)
    out = assemble_output(res.results)
    return out, res


def kernel(**inputs) -> np.ndarray:
    return _run(inputs)[0]


if __name__ == "__main__":
    rng = np.random.default_rng(0)
    inputs = {
        "attention_scores": rng.standard_normal((B, H, S, S), dtype=np.float32),
        "input_ids": rng.integers(0, CTX, size=(B, S)),
        "emb_W": rng.standard_normal((CTX, VD), dtype=np.float32) * 0.05,
        "ln_v_w": np.ones(VD, np.float32), "ln_v_b": np.zeros(VD, np.float32),
        "pW1": rng.standard_normal((VD, 2 * VD), dtype=np.float32) * 0.05,
        "pb1": rng.standard_normal(2 * VD, dtype=np.float32) * 0.05,
        "pW2": rng.standard_normal((2 * VD, H), dtype=np.float32) * 0.04,
        "pb2": rng.standard_normal(H, dtype=np.float32) * 0.04,
        "ln_c_w": np.ones(H + 1, np.float32), "ln_c_b": np.zeros(H + 1, np.float32),
        "fW1": rng.standard_normal((H + 1, FFN), dtype=np.float32) * 0.2,
        "fb1": rng.standard_normal(FFN, dtype=np.float32) * 0.2,
        "fW2": rng.standard_normal((FFN, FFN // 2), dtype=np.float32) * 0.04,
        "fb2": rng.standard_normal(FFN // 2, dtype=np.float32) * 0.04,
        "fW3": rng.standard_normal((FFN // 2, 1), dtype=np.float32) * 0.06,
        "fb3": rng.standard_normal(1, dtype=np.float32) * 0.06,
    }
    out = kernel(**inputs)
    print("kernel output", out.shape, out.dtype, float(np.abs(out).mean()))
